# revision 25
# baseline (speedup 1.0000x reference)
"""Trainium2 Bass kernel for ConvNAT (conv stem + 7x7 neighborhood attention).

Sharding: 8 cores = 4 batches x 2 H-halves. Bottom-half cores receive a
vertically flipped (and one-row-shifted) input plus kh-flipped conv weights so
that every core runs the IDENTICAL SPMD program computing output rows 0..27.

Per-core pipeline (all on-chip, fp32):
  im2col DMA -> conv matmul (K=27) + ReLU -> Q/K (CxPix layout) and V^T
  (PixxC layout, ones-augmented col for the softmax denominator) ->
  per row-pair: 4 banded QK^T matmuls -> exp (ACT) -> band-mask multiply ->
  4 AV matmuls (E^T as stationary) -> reciprocal-normalize -> DMA out.
Output is written as [pair, u, c] and transposed to NCHW on the host.
"""

import sys

for _p in ("/opt/trn_rl_repo",):
    if _p not in sys.path:
        sys.path.insert(0, _p)

import numpy as np

PLANE = 63 * 114       # padded x plane size
PLANE_AL = 32 * 2 * 114  # 7296, row-pair aligned plane allocation
H = 56            # conv output height/width
W = 56
C = 128           # channels
KSZ = 7           # attention window
NPAIR = 14        # row pairs per core (28 output rows)
KVROWS = 31       # K/V rows per core (rows 0..30 local)
NPIX_KV = KVROWS * W   # 1736
NPIX_Q = 28 * W        # 1568
XROWS = 63        # x shard rows (row 0 is the baked-in top pad row)
XCOLS = 114       # x shard cols (col 0 / col 113 are baked-in W pad)
SCALE = float(C) ** -0.5

# odd-aligned k-row-pair blocks {2g+1, 2g+2} for interior pairs plus
# even-aligned blocks {0,1},{2,3},{4,5},{6,7} for pairs 0 and 1.
V_BLOCK_STARTS = [0, 2, 4, 6] + [2 * g + 1 for g in range(15)]  # 19 blocks


def _pair_blocks(p):
    """Block indices into V_BLOCK_STARTS for pair p, and k-row starts."""
    if p < 2:
        idx = [0, 1, 2, 3]
    else:
        idx = [4 + (p - 2) + b for b in range(4)]
    return idx, [V_BLOCK_STARTS[i] for i in idx]


def _build_masks():
    s = np.arange(W)
    cs = np.clip(s - 3, 0, W - KSZ)
    t = np.arange(W)[:, None]
    colband = ((t >= cs[None, :]) & (t <= cs[None, :] + 6)).astype(np.float32)
    masks = {}
    for kind in ("edge", "int"):
        m = np.zeros((112, 448), np.float32)
        for bb in range(4):
            for jl in range(2):
                for h in range(2):
                    if kind == "edge":
                        rv = not (bb == 3 and jl == 1)
                    else:
                        rv = not ((bb == 3 and jl == 1 and h == 0)
                                  or (bb == 0 and jl == 0 and h == 1))
                    if rv:
                        m[jl * 56:(jl + 1) * 56,
                          bb * 112 + h * 56: bb * 112 + h * 56 + 56] = colband
        masks[kind] = m
    return masks["edge"], masks["int"]


_PROGRAM = None  # (nc,) cache — trace+compile once per process


def _build_program(has_vb):
    import concourse.bass as bass
    import concourse.mybir as mybir
    import concourse.tile as tile
    from concourse import bacc

    f32 = mybir.dt.float32
    nc = bacc.Bacc(None)

    xs = nc.declare_dram_parameter("xs", [9, PLANE_AL], f32, isOutput=False)
    cw = nc.declare_dram_parameter("cw", [9, 3, 128], f32, isOutput=False)
    cb = nc.declare_dram_parameter("cb", [128, 1], f32, isOutput=False)
    qw = nc.declare_dram_parameter("qw", [128, 128], f32, isOutput=False)
    qb = nc.declare_dram_parameter("qb", [128, 1], f32, isOutput=False)
    kw = nc.declare_dram_parameter("kw", [128, 128], f32, isOutput=False)
    kb = nc.declare_dram_parameter("kb", [128, 1], f32, isOutput=False)
    vw = nc.declare_dram_parameter("vw", [128, 128], f32, isOutput=False)
    vb = nc.declare_dram_parameter("vb", [1, 128], f32, isOutput=False)
    m_edge = nc.declare_dram_parameter("m_edge", [112, 448], f32, isOutput=False)
    m_int = nc.declare_dram_parameter("m_int", [112, 448], f32, isOutput=False)
    out2 = nc.declare_dram_parameter("out2", [NPAIR, 112, 128], f32, isOutput=True)

    Relu = mybir.ActivationFunctionType.Relu
    Ident = mybir.ActivationFunctionType.Identity
    Exp = mybir.ActivationFunctionType.Exp

    with tile.TileContext(nc) as tc:
        with (
            tc.tile_pool(name="singles", bufs=1) as singles,
            tc.tile_pool(name="big", bufs=1) as big,
            tc.tile_pool(name="work", bufs=3) as work,
            tc.tile_pool(name="psA", bufs=2, space="PSUM") as psA,
            tc.tile_pool(name="psL", bufs=4, space="PSUM") as psL,
            tc.tile_pool(name="psO", bufs=2, space="PSUM") as psO,
        ):
            # ---- load weights / biases / masks ----
            cw_t = singles.tile([9, 3, 128], f32, tag="cw")
            nc.sync.dma_start(out=cw_t[:], in_=cw[:])
            cb_t = singles.tile([128, 1], f32, tag="cb")
            nc.sync.dma_start(out=cb_t[:], in_=cb[:])
            qw_t = singles.tile([128, 128], f32, tag="qw")
            nc.sync.dma_start(out=qw_t[:], in_=qw[:])
            qb_t = singles.tile([128, 1], f32, tag="qb")
            nc.sync.dma_start(out=qb_t[:], in_=qb[:])
            kw_t = singles.tile([128, 128], f32, tag="kw")
            nc.sync.dma_start(out=kw_t[:], in_=kw[:])
            kb_t = singles.tile([128, 1], f32, tag="kb")
            nc.sync.dma_start(out=kb_t[:], in_=kb[:])
            vw_t = singles.tile([128, 128], f32, tag="vw")
            nc.sync.dma_start(out=vw_t[:], in_=vw[:])
            me_t = singles.tile([112, 448], f32, tag="me")
            nc.sync.dma_start(out=me_t[:], in_=m_edge[:])
            mi_t = singles.tile([112, 448], f32, tag="mi")
            nc.sync.dma_start(out=mi_t[:], in_=m_int[:])
            if has_vb:
                vb_t = singles.tile([112, 128], f32, tag="vb")
                nc.sync.dma_start(
                    out=vb_t[:],
                    in_=bass.AP(tensor=vb.tensor, offset=0,
                                ap=[[0, 112], [1, 128]]),
                )

            # ---- x planes, host-replicated 3x per ci with the kw shift baked
            # into each partition; kh becomes a PSUM-accumulated matmul over
            # row-shifted rhs APs. Load split 4 ways across DMA queues.
            x_sb = big.tile([9, PLANE_AL], f32, tag="xsb")
            qtr = PLANE_AL // 4
            for i in range(4):
                nc.sync.dma_start(out=x_sb[:, i * qtr:(i + 1) * qtr],
                                  in_=xs[:, i * qtr:(i + 1) * qtr])

            # ---- conv + relu -> f [128, 1736] ----
            f_sb = big.tile([128, NPIX_KV], f32, tag="f")
            conv_rows = [(0, 8), (8, 8), (16, 8), (24, 7)]
            x_sbr = x_sb[:].rearrange("p (h w) -> p h w", w=XCOLS)  # [9,64,114]
            for r0, nr in conv_rows:
                off, sz = r0 * W, nr * W
                ps = psA.tile([128, 448], f32, tag="ps")
                for kh in range(3):
                    h0 = 2 * r0 + kh
                    nc.tensor.matmul(ps[:, :sz], cw_t[:, kh, :],
                                     x_sbr[:, h0:h0 + 2 * nr - 1:2, 0:111:2],
                                     start=(kh == 0), stop=(kh == 2))
                nc.scalar.activation(f_sb[:, off:off + sz], ps[:, :sz],
                                     Relu, bias=cb_t[:])

            # ---- Q [128, 1568], K [128, 1736] ----
            q_sb = big.tile([128, NPIX_Q], f32, tag="q")
            for off in range(0, NPIX_Q, 392):
                ps = psA.tile([128, 448], f32, tag="ps")
                nc.tensor.matmul(ps[:, :392], qw_t[:], f_sb[:, off:off + 392],
                                 start=True, stop=True)
                nc.scalar.activation(q_sb[:, off:off + 392], ps[:, :392],
                                     Ident, bias=qb_t[:])
            k_sb = big.tile([128, NPIX_KV], f32, tag="k")
            for off in range(0, NPIX_KV, 434):
                ps = psA.tile([128, 448], f32, tag="ps")
                nc.tensor.matmul(ps[:, :434], kw_t[:], f_sb[:, off:off + 434],
                                 start=True, stop=True)
                nc.vector.tensor_scalar_add(k_sb[:, off:off + 434], ps[:, :434],
                                            kb_t[:])

            # ---- V^T blocks [112, 19*129], col 128 of each block = ones ----
            v_sb = big.tile([112, 19 * 129], f32, tag="v")
            for i, r0 in enumerate(V_BLOCK_STARTS):
                ps = psA.tile([112, 128], f32, tag="ps")
                nc.tensor.matmul(ps[:], f_sb[:, r0 * W: r0 * W + 112], vw_t[:],
                                 start=True, stop=True)
                if has_vb:
                    nc.vector.tensor_add(v_sb[:, i * 129: i * 129 + 128],
                                         ps[:], vb_t[:])
                else:
                    nc.vector.tensor_copy(v_sb[:, i * 129: i * 129 + 128], ps[:])
                nc.vector.memset(v_sb[:, i * 129 + 128: i * 129 + 129], 1.0)

            # ---- attention row pairs ----
            # QK^T runs k-block-major so one K LDWEIGHTS serves up to 4 pairs.
            # (block, [(pair, bb), ...]) schedule, in pair-completion order.
            sched = []        # (vblock_idx, [(p, bb), ...])
            for vi in range(len(V_BLOCK_STARTS)):
                uses = []
                for p in range(NPAIR):
                    vidx, _ = _pair_blocks(p)
                    for bb in range(4):
                        if vidx[bb] == vi:
                            uses.append((p, bb))
                if uses:
                    sched.append((vi, uses))

            psls = {}
            done = {p: 0 for p in range(NPAIR)}
            emitted = set()

            def finish_pair(p):
                psl = psls.pop(p)
                vidx, _ = _pair_blocks(p)
                e_sb = work.tile([112, 448], f32, tag="e")
                nc.scalar.activation(e_sb[:], psl[:], Exp, scale=SCALE)
                mask = me_t if p < 2 else mi_t
                nc.vector.tensor_mul(e_sb[:], e_sb[:], mask[:])
                pso = psO.tile([112, 129], f32, tag="pso")
                for bb in range(4):
                    vi = vidx[bb]
                    nc.tensor.matmul(
                        pso[:],
                        e_sb[:, bb * 112:(bb + 1) * 112],
                        v_sb[:, vi * 129:(vi + 1) * 129],
                        start=(bb == 0), stop=(bb == 3))
                recip = work.tile([112, 1], f32, tag="r")
                nc.vector.reciprocal(recip[:], pso[:, 128:129])
                out_n = work.tile([112, 128], f32, tag="on")
                nc.vector.tensor_scalar_mul(out_n[:], pso[:, 0:128], recip[:])
                nc.sync.dma_start(out=out2[p], in_=out_n[:])

            for vi, uses in sched:
                kr = V_BLOCK_STARTS[vi]
                for p, bb in uses:
                    if p not in psls:
                        psls[p] = psL.tile([112, 448], f32, name="psl", tag="psl")
                    nc.tensor.matmul(
                        psls[p][:, bb * 112:(bb + 1) * 112],
                        k_sb[:, kr * W: kr * W + 112],
                        q_sb[:, p * 112:(p + 1) * 112],
                        start=True, stop=True)
                    done[p] += 1
                    if done[p] == 4:
                        emitted.add(p)
                        finish_pair(p)

    nc.compile()
    return nc


def _prepare(x, conv_w, conv_b, q_w, q_b, k_w, k_b, v_w, v_b):
    """Build (and cache) the program; return (nc, in_maps)."""
    global _PROGRAM
    x = np.asarray(x, np.float32)
    conv_w = np.asarray(conv_w, np.float32)
    has_vb = bool(np.any(np.asarray(v_b) != 0))
    if _PROGRAM is None:
        _PROGRAM = _build_program(has_vb)
    nc = _PROGRAM

    m_edge, m_int = _build_masks()
    common = {
        "cb": np.ascontiguousarray(conv_b.reshape(128, 1), np.float32),
        "qw": np.ascontiguousarray(np.asarray(q_w, np.float32).T),
        "qb": np.ascontiguousarray(np.asarray(q_b, np.float32).reshape(128, 1)),
        "kw": np.ascontiguousarray(np.asarray(k_w, np.float32).T),
        "kb": np.ascontiguousarray(np.asarray(k_b, np.float32).reshape(128, 1)),
        "vw": np.ascontiguousarray(np.asarray(v_w, np.float32).T),
        "vb": np.ascontiguousarray(np.asarray(v_b, np.float32).reshape(1, 128)),
        "m_edge": m_edge,
        "m_int": m_int,
    }
    cw_top = np.ascontiguousarray(
        conv_w.transpose(1, 3, 2, 0).reshape(9, 3, 128))
    cw_bot = np.ascontiguousarray(
        conv_w[:, :, ::-1, :].transpose(1, 3, 2, 0).reshape(9, 3, 128))

    in_maps = []
    for core in range(8):
        b, half = core // 2, core % 2
        pad = np.zeros((3, XROWS, XCOLS), np.float32)
        if half == 0:
            pad[:, 1:63, 1:113] = x[b][:, 0:62, :]
            cwl = cw_top
        else:
            pad[:, 0:63, 1:113] = x[b][:, 111:48:-1, :]
            cwl = cw_bot
        flat = pad.reshape(3, -1)
        xrep = np.zeros((9, PLANE_AL), np.float32)
        for ci in range(3):
            for kwi in range(3):
                xrep[ci * 3 + kwi, :PLANE - kwi] = flat[ci, kwi:]
        in_maps.append({"xs": xrep, "cw": cwl, **common})
    return nc, in_maps


def _assemble(results):
    out = np.empty((4, 128, 56, 56), np.float32)
    for core in range(8):
        b, half = core // 2, core % 2
        a = results[core]["out2"]              # [14, 112, 128]
        a = a.reshape(14, 2, 56, 128).transpose(3, 0, 1, 2).reshape(128, 28, 56)
        if half == 0:
            out[b, :, 0:28, :] = a
        else:
            out[b, :, 28:56, :] = a[:, ::-1, :]
    return out


def kernel(**inputs):
    from concourse.bass_utils import run_bass_kernel_spmd

    nc, in_maps = _prepare(**inputs)
    res = run_bass_kernel_spmd(nc, in_maps, list(range(8)))
    return _assemble(res.results)


# revision 30
# speedup vs baseline: 1.2044x; 1.2044x over previous
"""Trainium2 Bass kernel for ConvNAT (conv stem + 7x7 neighborhood attention).

Sharding: 8 cores = 4 batches x 2 H-halves. Bottom-half cores receive a
vertically flipped (and one-row-shifted) input plus kh-flipped conv weights so
that every core runs the IDENTICAL SPMD program computing output rows 0..27.

Per-core pipeline (all on-chip, fp32):
  im2col DMA -> conv matmul (K=27) + ReLU -> Q/K (CxPix layout) and V^T
  (PixxC layout, ones-augmented col for the softmax denominator) ->
  per row-pair: 4 banded QK^T matmuls -> exp (ACT) -> band-mask multiply ->
  4 AV matmuls (E^T as stationary) -> reciprocal-normalize -> DMA out.
Output is written as [pair, u, c] and transposed to NCHW on the host.
"""

import sys

for _p in ("/opt/trn_rl_repo",):
    if _p not in sys.path:
        sys.path.insert(0, _p)

import numpy as np

PLANE = 63 * 114       # padded x plane size
PLANE_AL = 32 * 2 * 114  # 7296, row-pair aligned plane allocation
H = 56            # conv output height/width
W = 56
C = 128           # channels
KSZ = 7           # attention window
NPAIR = 14        # row pairs per core (28 output rows)
KVROWS = 31       # K/V rows per core (rows 0..30 local)
NPIX_KV = KVROWS * W   # 1736
NPIX_Q = 28 * W        # 1568
XROWS = 63        # x shard rows (row 0 is the baked-in top pad row)
XCOLS = 114       # x shard cols (col 0 / col 113 are baked-in W pad)
SCALE = float(C) ** -0.5

# odd-aligned k-row-pair blocks {2g+1, 2g+2} for interior pairs plus
# even-aligned blocks {0,1},{2,3},{4,5},{6,7} for pairs 0 and 1.
V_BLOCK_STARTS = [0, 2, 4, 6] + [2 * g + 1 for g in range(15)]  # 19 blocks


def _pair_blocks(p):
    """Block indices into V_BLOCK_STARTS for pair p, and k-row starts."""
    if p < 2:
        idx = [0, 1, 2, 3]
    else:
        idx = [4 + (p - 2) + b for b in range(4)]
    return idx, [V_BLOCK_STARTS[i] for i in idx]


def _build_masks():
    s = np.arange(W)
    cs = np.clip(s - 3, 0, W - KSZ)
    t = np.arange(W)[:, None]
    colband = ((t >= cs[None, :]) & (t <= cs[None, :] + 6)).astype(np.float32)
    masks = {}
    for kind in ("edge", "int"):
        m = np.zeros((112, 448), np.float32)
        for bb in range(4):
            for jl in range(2):
                for h in range(2):
                    if kind == "edge":
                        rv = not (bb == 3 and jl == 1)
                    else:
                        rv = not ((bb == 3 and jl == 1 and h == 0)
                                  or (bb == 0 and jl == 0 and h == 1))
                    if rv:
                        m[jl * 56:(jl + 1) * 56,
                          bb * 112 + h * 56: bb * 112 + h * 56 + 56] = colband
        masks[kind] = m
    return masks["edge"], masks["int"]


_PROGRAM = None  # (nc,) cache — trace+compile once per process


def _build_program(has_vb):
    import concourse.bass as bass
    import concourse.mybir as mybir
    import concourse.tile as tile
    from concourse import bacc

    f32 = mybir.dt.float32
    nc = bacc.Bacc(None)

    xs = nc.declare_dram_parameter("xs", [27, NPIX_KV], f32, isOutput=False)
    cw = nc.declare_dram_parameter("cw", [27, 128], f32, isOutput=False)
    cb = nc.declare_dram_parameter("cb", [128, 1], f32, isOutput=False)
    qw = nc.declare_dram_parameter("qw", [128, 128], f32, isOutput=False)
    qb = nc.declare_dram_parameter("qb", [128, 1], f32, isOutput=False)
    kw = nc.declare_dram_parameter("kw", [128, 128], f32, isOutput=False)
    kb = nc.declare_dram_parameter("kb", [128, 1], f32, isOutput=False)
    vw = nc.declare_dram_parameter("vw", [128, 128], f32, isOutput=False)
    vb = nc.declare_dram_parameter("vb", [1, 128], f32, isOutput=False)
    m_edge = nc.declare_dram_parameter("m_edge", [112, 448], f32, isOutput=False)
    m_int = nc.declare_dram_parameter("m_int", [112, 448], f32, isOutput=False)
    out2 = nc.declare_dram_parameter("out2", [NPAIR, 112, 128], f32, isOutput=True)

    Relu = mybir.ActivationFunctionType.Relu
    Ident = mybir.ActivationFunctionType.Identity
    Exp = mybir.ActivationFunctionType.Exp

    with tile.TileContext(nc) as tc:
        with (
            tc.tile_pool(name="singles", bufs=1) as singles,
            tc.tile_pool(name="big", bufs=1) as big,
            tc.tile_pool(name="work", bufs=3) as work,
            tc.tile_pool(name="psA", bufs=2, space="PSUM") as psA,
            tc.tile_pool(name="psL", bufs=4, space="PSUM") as psL,
            tc.tile_pool(name="psO", bufs=2, space="PSUM") as psO,
        ):
            # ---- load weights / biases / masks ----
            cw_t = singles.tile([27, 128], f32, tag="cw")
            nc.sync.dma_start(out=cw_t[:], in_=cw[:])
            cb_t = singles.tile([128, 1], f32, tag="cb")
            nc.sync.dma_start(out=cb_t[:], in_=cb[:])
            qw_t = singles.tile([128, 128], f32, tag="qw")
            nc.sync.dma_start(out=qw_t[:], in_=qw[:])
            qb_t = singles.tile([128, 1], f32, tag="qb")
            nc.sync.dma_start(out=qb_t[:], in_=qb[:])
            kw_t = singles.tile([128, 128], f32, tag="kw")
            nc.sync.dma_start(out=kw_t[:], in_=kw[:])
            kb_t = singles.tile([128, 1], f32, tag="kb")
            nc.sync.dma_start(out=kb_t[:], in_=kb[:])
            vw_t = singles.tile([128, 128], f32, tag="vw")
            nc.sync.dma_start(out=vw_t[:], in_=vw[:])
            me_t = singles.tile([112, 448], f32, tag="me")
            nc.sync.dma_start(out=me_t[:], in_=m_edge[:])
            mi_t = singles.tile([112, 448], f32, tag="mi")
            nc.sync.dma_start(out=mi_t[:], in_=m_int[:])
            if has_vb:
                vb_t = singles.tile([112, 128], f32, tag="vb")
                nc.sync.dma_start(
                    out=vb_t[:],
                    in_=bass.AP(tensor=vb.tensor, offset=0,
                                ap=[[0, 112], [1, 128]]),
                )

            # ---- im2col planes, decimated on the host: xs[q, (r, s)] =
            # x[ci, 2r+kh-1, 2s+kw-1] (pad baked in). Conv rhs is contiguous.
            x_sb = big.tile([27, NPIX_KV], f32, tag="xsb")
            half_px = NPIX_KV // 2
            nc.sync.dma_start(out=x_sb[:, :half_px], in_=xs[:, :half_px])
            nc.sync.dma_start(out=x_sb[:, half_px:], in_=xs[:, half_px:])

            # ---- conv + relu -> f [128, 1736] ----
            f_sb = big.tile([128, NPIX_KV], f32, tag="f")
            conv_rows = [(0, 8), (8, 8), (16, 8), (24, 7)]
            for r0, nr in conv_rows:
                off, sz = r0 * W, nr * W
                ps = psA.tile([128, 448], f32, tag="ps")
                nc.tensor.matmul(ps[:, :sz], cw_t[:], x_sb[:, off:off + sz],
                                 start=True, stop=True)
                nc.scalar.activation(f_sb[:, off:off + sz], ps[:, :sz],
                                     Relu, bias=cb_t[:])

            # ---- Q [128, 1568], K [128, 1736] ----
            q_sb = big.tile([128, NPIX_Q], f32, tag="q")
            for off in range(0, NPIX_Q, 392):
                ps = psA.tile([128, 448], f32, tag="ps")
                nc.tensor.matmul(ps[:, :392], qw_t[:], f_sb[:, off:off + 392],
                                 start=True, stop=True)
                nc.scalar.activation(q_sb[:, off:off + 392], ps[:, :392],
                                     Ident, bias=qb_t[:])
            k_sb = big.tile([128, NPIX_KV], f32, tag="k")
            for off in range(0, NPIX_KV, 434):
                ps = psA.tile([128, 448], f32, tag="ps")
                nc.tensor.matmul(ps[:, :434], kw_t[:], f_sb[:, off:off + 434],
                                 start=True, stop=True)
                nc.vector.tensor_scalar_add(k_sb[:, off:off + 434], ps[:, :434],
                                            kb_t[:])

            # ---- V^T blocks [112, 19*129], col 128 of each block = ones ----
            v_sb = big.tile([112, 19 * 129], f32, tag="v")
            for i, r0 in enumerate(V_BLOCK_STARTS):
                ps = psA.tile([112, 128], f32, tag="ps")
                nc.tensor.matmul(ps[:], f_sb[:, r0 * W: r0 * W + 112], vw_t[:],
                                 start=True, stop=True)
                if has_vb:
                    nc.vector.tensor_add(v_sb[:, i * 129: i * 129 + 128],
                                         ps[:], vb_t[:])
                else:
                    nc.vector.tensor_copy(v_sb[:, i * 129: i * 129 + 128], ps[:])
                nc.vector.memset(v_sb[:, i * 129 + 128: i * 129 + 129], 1.0)

            # ---- attention row pairs ----
            # QK^T runs k-block-major so one K LDWEIGHTS serves up to 4 pairs.
            # (block, [(pair, bb), ...]) schedule, in pair-completion order.
            sched = []        # (vblock_idx, [(p, bb), ...])
            for vi in range(len(V_BLOCK_STARTS)):
                uses = []
                for p in range(NPAIR):
                    vidx, _ = _pair_blocks(p)
                    for bb in range(4):
                        if vidx[bb] == vi:
                            uses.append((p, bb))
                if uses:
                    sched.append((vi, uses))

            psls = {}
            done = {p: 0 for p in range(NPAIR)}
            emitted = set()

            def finish_pair(p):
                psl = psls.pop(p)
                vidx, _ = _pair_blocks(p)
                e_sb = work.tile([112, 448], f32, tag="e")
                nc.scalar.activation(e_sb[:], psl[:], Exp, scale=SCALE)
                mask = me_t if p < 2 else mi_t
                nc.vector.tensor_mul(e_sb[:], e_sb[:], mask[:])
                pso = psO.tile([112, 129], f32, tag="pso")
                for bb in range(4):
                    vi = vidx[bb]
                    nc.tensor.matmul(
                        pso[:],
                        e_sb[:, bb * 112:(bb + 1) * 112],
                        v_sb[:, vi * 129:(vi + 1) * 129],
                        start=(bb == 0), stop=(bb == 3))
                recip = work.tile([112, 1], f32, tag="r")
                nc.vector.reciprocal(recip[:], pso[:, 128:129])
                out_n = work.tile([112, 128], f32, tag="on")
                nc.vector.tensor_scalar_mul(out_n[:], pso[:, 0:128], recip[:])
                nc.sync.dma_start(out=out2[p], in_=out_n[:])

            for vi, uses in sched:
                kr = V_BLOCK_STARTS[vi]
                for p, bb in uses:
                    if p not in psls:
                        psls[p] = psL.tile([112, 448], f32, name="psl", tag="psl")
                    nc.tensor.matmul(
                        psls[p][:, bb * 112:(bb + 1) * 112],
                        k_sb[:, kr * W: kr * W + 112],
                        q_sb[:, p * 112:(p + 1) * 112],
                        start=True, stop=True)
                    done[p] += 1
                    if done[p] == 4:
                        emitted.add(p)
                        finish_pair(p)

    nc.compile()
    return nc


def _prepare(x, conv_w, conv_b, q_w, q_b, k_w, k_b, v_w, v_b):
    """Build (and cache) the program; return (nc, in_maps)."""
    global _PROGRAM
    x = np.asarray(x, np.float32)
    conv_w = np.asarray(conv_w, np.float32)
    has_vb = bool(np.any(np.asarray(v_b) != 0))
    if _PROGRAM is None:
        _PROGRAM = _build_program(has_vb)
    nc = _PROGRAM

    m_edge, m_int = _build_masks()
    common = {
        "cb": np.ascontiguousarray(conv_b.reshape(128, 1), np.float32),
        "qw": np.ascontiguousarray(np.asarray(q_w, np.float32).T),
        "qb": np.ascontiguousarray(np.asarray(q_b, np.float32).reshape(128, 1)),
        "kw": np.ascontiguousarray(np.asarray(k_w, np.float32).T),
        "kb": np.ascontiguousarray(np.asarray(k_b, np.float32).reshape(128, 1)),
        "vw": np.ascontiguousarray(np.asarray(v_w, np.float32).T),
        "vb": np.ascontiguousarray(np.asarray(v_b, np.float32).reshape(1, 128)),
        "m_edge": m_edge,
        "m_int": m_int,
    }
    cw_top = np.ascontiguousarray(
        conv_w.transpose(1, 2, 3, 0).reshape(27, 128))
    cw_bot = np.ascontiguousarray(
        conv_w[:, :, ::-1, :].transpose(1, 2, 3, 0).reshape(27, 128))

    in_maps = []
    for core in range(8):
        b, half = core // 2, core % 2
        pad = np.zeros((3, XROWS, XCOLS), np.float32)
        if half == 0:
            pad[:, 1:63, 1:113] = x[b][:, 0:62, :]
            cwl = cw_top
        else:
            pad[:, 0:63, 1:113] = x[b][:, 111:48:-1, :]
            cwl = cw_bot
        xrep = np.empty((27, NPIX_KV), np.float32)
        for ci in range(3):
            for kh in range(3):
                for kwi in range(3):
                    xrep[ci * 9 + kh * 3 + kwi] = (
                        pad[ci, kh:kh + 61:2, kwi:kwi + 111:2].reshape(-1))
        in_maps.append({"xs": xrep, "cw": cwl, **common})
    return nc, in_maps


def _assemble(results):
    out = np.empty((4, 128, 56, 56), np.float32)
    for core in range(8):
        b, half = core // 2, core % 2
        a = results[core]["out2"]              # [14, 112, 128]
        a = a.reshape(14, 2, 56, 128).transpose(3, 0, 1, 2).reshape(128, 28, 56)
        if half == 0:
            out[b, :, 0:28, :] = a
        else:
            out[b, :, 28:56, :] = a[:, ::-1, :]
    return out


def kernel(**inputs):
    from concourse.bass_utils import run_bass_kernel_spmd

    nc, in_maps = _prepare(**inputs)
    res = run_bass_kernel_spmd(nc, in_maps, list(range(8)))
    return _assemble(res.results)
